# revision 16
# baseline (speedup 1.0000x reference)
"""Trainium2 Bass kernel for DCKModule (involution / dynamic conv kernel).

Math (per batch image, all fp32):
  x  = relu(W1 @ guide * bn_scale + bn_bias)        # (64, 9216)
  df = W2 @ x                                       # (784, 9216) = (16 groups * 49 taps)
  out[g,gc,p] = sum_k df[g,k,p] * fpad[g,gc, p+off_k] + feature[g,gc,p]

Mapping: data-parallel over batch (1 image per NeuronCore, 8 cores).
BN scale folded into W1 host-side. The 16x broadcast of df over group
channels is done for free on the TensorEngine by replicating rows of W2
(W2exp trick): for each tap k, Dk = W2exp_k @ x lands in PSUM already
broadcast to all 256 channels. VectorE then does acc += Dk * F_shift.
"""

import numpy as np

import concourse.bass as bass
import concourse.mybir as mybir
import concourse.tile as tile
from concourse import bacc, bass_utils

B, C, H, W = 8, 256, 96, 96
K7, PAD, G, GC, R = 7, 3, 16, 16, 64
HP = H + 2 * PAD          # 102
PIX = H * W               # 9216
BN_EPS = 1e-5
RBLK = 8                  # output rows per pipeline block
NBLK = H // RBLK          # 12
BLKPIX = RBLK * W         # 768
SUB = 384                 # matmul free-dim chunk (<=512, = 4 rows)
SUBROWS = SUB // W        # 4
NSUB = BLKPIX // SUB      # 2

F32 = mybir.dt.float32
TRACE = False

_CACHE = {}


def _build_nc():
    nc = bacc.Bacc(None, target_bir_lowering=False)
    fm_d = nc.dram_tensor("fm", [C, HP * HP], F32, kind="ExternalInput")
    gm_d = nc.dram_tensor("gm", [C, PIX], F32, kind="ExternalInput")
    w1_d = nc.dram_tensor("w1pt", [C, R], F32, kind="ExternalInput")
    bias_d = nc.dram_tensor("bias", [R, 1], F32, kind="ExternalInput")
    w2_d = nc.dram_tensor("w2et", [R, 49 * C], F32, kind="ExternalInput")
    out_d = nc.dram_tensor("out", [C, PIX], F32, kind="ExternalOutput")

    with tile.TileContext(nc) as tc:
        with tc.tile_pool(name="persist", bufs=1) as persist, \
             tc.tile_pool(name="gpool", bufs=2) as gpool, \
             tc.tile_pool(name="xpool", bufs=2) as xpool, \
             tc.tile_pool(name="accpool", bufs=2) as accpool, \
             tc.tile_pool(name="prodpool", bufs=4) as prodpool, \
             tc.tile_pool(name="psx", bufs=1, space="PSUM") as psx, \
             tc.tile_pool(name="psdk", bufs=3, space="PSUM") as psdk:

            fpad = [persist.tile([128, HP * HP], F32, tag=f"fpad{ct}", name=f"fpad{ct}")
                    for ct in range(2)]
            w1_sb = persist.tile([128, 2 * R], F32, tag="w1", name="w1sb")
            bias_sb = persist.tile([R, 1], F32, tag="bias", name="biassb")
            w2_sb = persist.tile([R, 49 * C], F32, tag="w2", name="w2sb")

            for ct in range(2):
                nc.gpsimd.dma_start(
                    out=fpad[ct][:],
                    in_=fm_d[ct * 128:(ct + 1) * 128, :])
            for ck in range(2):
                nc.gpsimd.dma_start(out=w1_sb[:, ck * R:(ck + 1) * R],
                                  in_=w1_d[ck * 128:(ck + 1) * 128, :])
            nc.gpsimd.dma_start(out=bias_sb[:], in_=bias_d[:])
            nc.gpsimd.dma_start(out=w2_sb[:], in_=w2_d[:])

            obs = psx.tile([1, 2], F32, tag="obs", name="obs", bufs=1)
            nc.tensor.matmul(obs[:, 0:1], w1_sb[:, 0:1], w1_sb[:, 0:1],
                             start=True, stop=True)
            nc.tensor.matmul(obs[:, 0:1], w1_sb[:, R:R + 1],
                             w1_sb[:, R:R + 1], start=True, stop=True)
            nc.tensor.matmul(obs[:, 1:2], w2_sb[:, 0:1], w2_sb[:, 0:1],
                             start=True, stop=True)
            vobs = persist.tile([128, 3], F32, tag="vobs", name="vobs")
            nc.vector.tensor_copy(vobs[:R, 0:1], bias_sb[:])
            nc.vector.tensor_copy(vobs[:, 1:2], fpad[0][:, 0:1])
            nc.vector.tensor_copy(vobs[:, 2:3], fpad[1][:, 0:1])

            fviews = [fpad[ct][:].rearrange("p (r j) -> p r j", j=HP)
                      for ct in range(2)]

            for blk in range(NBLK):
                r0 = blk * RBLK
                g_sb = [gpool.tile([128, BLKPIX], F32, tag=f"g{ct}", name=f"gsb{ct}")
                        for ct in range(2)]
                for ct in range(2):
                    nc.sync.dma_start(
                        out=g_sb[ct][:],
                        in_=gm_d[ct * 128:(ct + 1) * 128,
                                 r0 * W:(r0 + RBLK) * W])

                x_sb = xpool.tile([R, BLKPIX], F32, tag="x", name="xsb")
                for s in range(NSUB):
                    px = psx.tile([R, SUB], F32, tag="px", name="px")
                    for ck in range(2):
                        nc.tensor.matmul(
                            px[:], w1_sb[:, ck * R:(ck + 1) * R],
                            g_sb[ck][:, s * SUB:(s + 1) * SUB],
                            start=(ck == 0), stop=(ck == 1))
                    nc.vector.tensor_scalar(
                        x_sb[:, s * SUB:(s + 1) * SUB], px[:],
                        bias_sb[:], 0.0,
                        mybir.AluOpType.add, mybir.AluOpType.max)

                acc = [accpool.tile([128, BLKPIX], F32, tag=f"acc{ct}", name=f"acc{ct}")
                       for ct in range(2)]

                for k in range(49):
                    di, dj = divmod(k, K7)
                    for ct in range(2):
                        # dk spans 2 PSUM banks: rows 0-3 at offset 0,
                        # rows 4-7 at offset 512 (each matmul <= 1 bank)
                        dk = psdk.tile([128, 2 * 512], F32, tag="dk", name="dk")
                        for s in range(NSUB):
                            nc.tensor.matmul(
                                dk[:, s * 512:s * 512 + SUB],
                                w2_sb[:, k * C + ct * 128:k * C + ct * 128 + 128],
                                x_sb[:, s * SUB:(s + 1) * SUB],
                                start=True, stop=True)
                        dkv = dk[:].rearrange("p (s q) -> p s q", s=2)[:, :, 0:SUB]                                    .rearrange("p s (r j) -> p s r j", j=W)
                        fsl = fviews[ct][:, r0 + di:r0 + di + RBLK, dj:dj + W]                             .rearrange("p (s r) j -> p s r j", s=NSUB)
                        accv = acc[ct][:].rearrange(
                            "p (s r j) -> p s r j", s=NSUB, j=W)
                        if k == 0:
                            nc.vector.tensor_tensor(
                                accv, dkv, fsl, mybir.AluOpType.mult)
                        else:
                            prod = prodpool.tile([128, BLKPIX], F32,
                                                 tag="prod", name="prod")
                            prodv = prod[:].rearrange(
                                "p (s r j) -> p s r j", s=NSUB, j=W)
                            nc.vector.tensor_tensor(
                                prodv, dkv, fsl, mybir.AluOpType.mult)
                            nc.vector.tensor_tensor(
                                acc[ct][:], acc[ct][:], prod[:],
                                mybir.AluOpType.add)

                for ct in range(2):
                    # residual
                    nc.vector.tensor_tensor(
                        acc[ct][:].rearrange("p (r j) -> p r j", j=W),
                        acc[ct][:].rearrange("p (r j) -> p r j", j=W),
                        fviews[ct][:, PAD + r0:PAD + r0 + RBLK, PAD:PAD + W],
                        mybir.AluOpType.add)
                    nc.sync.dma_start(
                        out=out_d[ct * 128:(ct + 1) * 128,
                                  r0 * W:(r0 + RBLK) * W],
                        in_=acc[ct][:])
    if not nc.is_finalized():
        nc.finalize()
    return nc


def _host_weights(W1, bn_gamma, bn_beta, bn_mean, bn_var, W2):
    inv = bn_gamma / np.sqrt(bn_var + BN_EPS)
    W1p = (W1 * inv[:, None]).astype(np.float32)          # (64, 256)
    w1pt = np.ascontiguousarray(W1p.T)                     # (256, 64)
    bias = (bn_beta - bn_mean * inv).astype(np.float32).reshape(R, 1)
    W2r = W2.reshape(G, 49, R)                             # [g, k, o]
    w2et = np.ascontiguousarray(
        np.repeat(W2r.transpose(2, 1, 0)[:, :, :, None], GC, axis=3)
        .reshape(R, 49 * C)).astype(np.float32)            # [o, k*256 + c]
    return w1pt, bias, w2et


def kernel(feature_map, guide_map, W1, bn_gamma, bn_beta, bn_mean, bn_var, W2):
    fm4 = np.asarray(feature_map, np.float32).reshape(B, C, H, W)
    fm = np.ascontiguousarray(
        np.pad(fm4, ((0, 0), (0, 0), (PAD, PAD), (PAD, PAD)))
        .reshape(B, C, HP * HP))
    gm = np.ascontiguousarray(np.asarray(guide_map, np.float32)
                              .reshape(B, C, PIX))
    w1pt, bias, w2et = _host_weights(
        np.asarray(W1, np.float32), np.asarray(bn_gamma, np.float32),
        np.asarray(bn_beta, np.float32), np.asarray(bn_mean, np.float32),
        np.asarray(bn_var, np.float32), np.asarray(W2, np.float32))

    if "nc" not in _CACHE:
        _CACHE["nc"] = _build_nc()
    nc = _CACHE["nc"]

    in_maps = [dict(fm=fm[i], gm=gm[i], w1pt=w1pt, bias=bias, w2et=w2et)
               for i in range(B)]
    _CACHE["in_maps"] = in_maps
    res = bass_utils.run_bass_kernel_spmd(
        nc, in_maps, core_ids=list(range(B)), trace=TRACE)
    _CACHE["last"] = res
    out = np.stack([r["out"] for r in res.results], axis=0)
    return out.reshape(B, C, H, W)


# revision 20
# speedup vs baseline: 32.8108x; 32.8108x over previous
"""Trainium2 Bass kernel for DCKModule (involution / dynamic conv kernel).

Math (per batch image, all fp32):
  x  = relu(W1 @ guide * bn_scale + bn_bias)        # (64, 9216)
  df = W2 @ x                                       # (784, 9216) = (16 groups * 49 taps)
  out[g,gc,p] = sum_k df[g,k,p] * fpad[g,gc, p+off_k] + feature[g,gc,p]

Mapping: data-parallel over batch (1 image per NeuronCore, 8 cores).
BN scale folded into W1 host-side; feature map padded host-side. The 16x
broadcast of df over group channels is done for free on the TensorEngine
by replicating rows of W2 (W2exp trick): for each tap k,
Dk = W2exp_k @ x lands in PSUM already broadcast to all 256 channels.
VectorE then does acc += Dk * F_shift (one 12-row mult + add per tap).

The kernel is VectorE-bound (~2.1 ms/core predicted by TimelineSim):
fp32 tensor_tensor runs at 1x (128 lanes @ 0.96 GHz) and the involution
needs 49 taps x 256 ch x 9216 px multiply-adds = 1.8M DVE cycles floor.
PE (matmuls), ScalarE (bias+relu) and DMA all hide behind it.

Toolchain notes (hard-won):
- Must build with bacc.Bacc + nc.finalize(): Bacc.compile() splits
  semaphore waits to the 1-wait-per-instruction HW limit; raw bass.Bass
  dies in walrus with "Too many sync wait commands".
- Big preload DMAs go on gpsimd (SWDGE, one queue sem each); tiny
  observer matmuls make PE consume those sems one at a time so no
  Matmult ever needs two DMA-queue waits.
- PSUM budget: 6 banks dk (2 bufs x 3 banks) + 1 px + 1 obs = 8.
"""

import numpy as np

import concourse.bass as bass
import concourse.mybir as mybir
import concourse.tile as tile
from concourse import bacc, bass_utils

B, C, H, W = 8, 256, 96, 96
K7, PAD, G, GC, R = 7, 3, 16, 16, 64
HP = H + 2 * PAD          # 102
PIX = H * W               # 9216
BN_EPS = 1e-5
RBLK = 12                 # output rows per pipeline block
NBLK = H // RBLK          # 12
BLKPIX = RBLK * W         # 768
SUB = 384                 # matmul free-dim chunk (<=512, = 4 rows)
SUBROWS = SUB // W        # 4
NSUB = BLKPIX // SUB      # 2

F32 = mybir.dt.float32
TRACE = False

_CACHE = {}


def _build_nc():
    nc = bacc.Bacc(None, target_bir_lowering=False)
    fm_d = nc.dram_tensor("fm", [C, HP * HP], F32, kind="ExternalInput")
    gm_d = nc.dram_tensor("gm", [C, PIX], F32, kind="ExternalInput")
    w1_d = nc.dram_tensor("w1pt", [C, R], F32, kind="ExternalInput")
    bias_d = nc.dram_tensor("bias", [R, 1], F32, kind="ExternalInput")
    w2_d = nc.dram_tensor("w2et", [R, 49 * C], F32, kind="ExternalInput")
    out_d = nc.dram_tensor("out", [C, PIX], F32, kind="ExternalOutput")

    with tile.TileContext(nc) as tc:
        with tc.tile_pool(name="persist", bufs=1) as persist, \
             tc.tile_pool(name="gpool", bufs=2) as gpool, \
             tc.tile_pool(name="xpool", bufs=2) as xpool, \
             tc.tile_pool(name="accpool", bufs=2) as accpool, \
             tc.tile_pool(name="prodpool", bufs=4) as prodpool, \
             tc.tile_pool(name="psx", bufs=1, space="PSUM") as psx, \
             tc.tile_pool(name="psdk", bufs=2, space="PSUM") as psdk:

            fpad = [persist.tile([128, HP * HP], F32, tag=f"fpad{ct}", name=f"fpad{ct}")
                    for ct in range(2)]
            w1_sb = persist.tile([128, 2 * R], F32, tag="w1", name="w1sb")
            bias_sb = persist.tile([R, 1], F32, tag="bias", name="biassb")
            w2_sb = persist.tile([R, 49 * C], F32, tag="w2", name="w2sb")

            for ct in range(2):
                nc.gpsimd.dma_start(
                    out=fpad[ct][:],
                    in_=fm_d[ct * 128:(ct + 1) * 128, :])
            for ck in range(2):
                nc.gpsimd.dma_start(out=w1_sb[:, ck * R:(ck + 1) * R],
                                  in_=w1_d[ck * 128:(ck + 1) * 128, :])
            nc.gpsimd.dma_start(out=bias_sb[:], in_=bias_d[:])
            nc.gpsimd.dma_start(out=w2_sb[:], in_=w2_d[:])

            obs = psx.tile([1, 2], F32, tag="obs", name="obs", bufs=1)
            nc.tensor.matmul(obs[:, 0:1], w1_sb[:, 0:1], w1_sb[:, 0:1],
                             start=True, stop=True)
            nc.tensor.matmul(obs[:, 0:1], w1_sb[:, R:R + 1],
                             w1_sb[:, R:R + 1], start=True, stop=True)
            nc.tensor.matmul(obs[:, 1:2], w2_sb[:, 0:1], w2_sb[:, 0:1],
                             start=True, stop=True)
            vobs = persist.tile([128, 3], F32, tag="vobs", name="vobs")
            nc.vector.tensor_copy(vobs[:R, 0:1], bias_sb[:])
            nc.vector.tensor_copy(vobs[:, 1:2], fpad[0][:, 0:1])
            nc.vector.tensor_copy(vobs[:, 2:3], fpad[1][:, 0:1])

            fviews = [fpad[ct][:].rearrange("p (r j) -> p r j", j=HP)
                      for ct in range(2)]

            for blk in range(NBLK):
                r0 = blk * RBLK
                g_sb = [gpool.tile([128, BLKPIX], F32, tag=f"g{ct}", name=f"gsb{ct}")
                        for ct in range(2)]
                for ct in range(2):
                    nc.sync.dma_start(
                        out=g_sb[ct][:],
                        in_=gm_d[ct * 128:(ct + 1) * 128,
                                 r0 * W:(r0 + RBLK) * W])

                x_sb = xpool.tile([R, BLKPIX], F32, tag="x", name="xsb")
                for s in range(NSUB):
                    px = psx.tile([R, SUB], F32, tag="px", name="px")
                    for ck in range(2):
                        nc.tensor.matmul(
                            px[:], w1_sb[:, ck * R:(ck + 1) * R],
                            g_sb[ck][:, s * SUB:(s + 1) * SUB],
                            start=(ck == 0), stop=(ck == 1))
                    nc.scalar.activation(
                        x_sb[:, s * SUB:(s + 1) * SUB], px[:],
                        mybir.ActivationFunctionType.Relu,
                        bias=bias_sb[:], scale=1.0)

                acc = [accpool.tile([128, BLKPIX], F32, tag=f"acc{ct}", name=f"acc{ct}")
                       for ct in range(2)]

                for k in range(49):
                    di, dj = divmod(k, K7)
                    for ct in range(2):
                        # dk spans NSUB PSUM banks, one 4-row (384 elem)
                        # matmul per bank (N<=512/bank); one fat DVE
                        # mult+add per tap then covers all 12 rows
                        dk = psdk.tile([128, NSUB * 512], F32, tag="dk", name="dk")
                        for s in range(NSUB):
                            nc.tensor.matmul(
                                dk[:, s * 512:s * 512 + SUB],
                                w2_sb[:, k * C + ct * 128:k * C + ct * 128 + 128],
                                x_sb[:, s * SUB:(s + 1) * SUB],
                                start=True, stop=True)
                        dkv = dk[:].rearrange("p (s q) -> p s q", s=NSUB)[:, :, 0:SUB]                                    .rearrange("p s (r j) -> p s r j", j=W)
                        fsl = fviews[ct][:, r0 + di:r0 + di + RBLK, dj:dj + W]                             .rearrange("p (s r) j -> p s r j", s=NSUB)
                        accv = acc[ct][:].rearrange(
                            "p (s r j) -> p s r j", s=NSUB, j=W)
                        if k == 0:
                            nc.vector.tensor_tensor(
                                accv, dkv, fsl, mybir.AluOpType.mult)
                        else:
                            prod = prodpool.tile([128, BLKPIX], F32,
                                                 tag="prod", name="prod")
                            prodv = prod[:].rearrange(
                                "p (s r j) -> p s r j", s=NSUB, j=W)
                            nc.vector.tensor_tensor(
                                prodv, dkv, fsl, mybir.AluOpType.mult)
                            nc.vector.tensor_tensor(
                                acc[ct][:], acc[ct][:], prod[:],
                                mybir.AluOpType.add)

                for ct in range(2):
                    # residual
                    nc.vector.tensor_tensor(
                        acc[ct][:].rearrange("p (r j) -> p r j", j=W),
                        acc[ct][:].rearrange("p (r j) -> p r j", j=W),
                        fviews[ct][:, PAD + r0:PAD + r0 + RBLK, PAD:PAD + W],
                        mybir.AluOpType.add)
                    nc.sync.dma_start(
                        out=out_d[ct * 128:(ct + 1) * 128,
                                  r0 * W:(r0 + RBLK) * W],
                        in_=acc[ct][:])
    if not nc.is_finalized():
        nc.finalize()
    return nc


def _host_weights(W1, bn_gamma, bn_beta, bn_mean, bn_var, W2):
    inv = bn_gamma / np.sqrt(bn_var + BN_EPS)
    W1p = (W1 * inv[:, None]).astype(np.float32)          # (64, 256)
    w1pt = np.ascontiguousarray(W1p.T)                     # (256, 64)
    bias = (bn_beta - bn_mean * inv).astype(np.float32).reshape(R, 1)
    W2r = W2.reshape(G, 49, R)                             # [g, k, o]
    w2et = np.ascontiguousarray(
        np.repeat(W2r.transpose(2, 1, 0)[:, :, :, None], GC, axis=3)
        .reshape(R, 49 * C)).astype(np.float32)            # [o, k*256 + c]
    return w1pt, bias, w2et


def kernel(feature_map, guide_map, W1, bn_gamma, bn_beta, bn_mean, bn_var, W2):
    fm4 = np.asarray(feature_map, np.float32).reshape(B, C, H, W)
    fm = np.ascontiguousarray(
        np.pad(fm4, ((0, 0), (0, 0), (PAD, PAD), (PAD, PAD)))
        .reshape(B, C, HP * HP))
    gm = np.ascontiguousarray(np.asarray(guide_map, np.float32)
                              .reshape(B, C, PIX))
    w1pt, bias, w2et = _host_weights(
        np.asarray(W1, np.float32), np.asarray(bn_gamma, np.float32),
        np.asarray(bn_beta, np.float32), np.asarray(bn_mean, np.float32),
        np.asarray(bn_var, np.float32), np.asarray(W2, np.float32))

    if "nc" not in _CACHE:
        _CACHE["nc"] = _build_nc()
    nc = _CACHE["nc"]

    in_maps = [dict(fm=fm[i], gm=gm[i], w1pt=w1pt, bias=bias, w2et=w2et)
               for i in range(B)]
    _CACHE["in_maps"] = in_maps
    res = bass_utils.run_bass_kernel_spmd(
        nc, in_maps, core_ids=list(range(B)), trace=TRACE)
    _CACHE["last"] = res
    out = np.stack([r["out"] for r in res.results], axis=0)
    return out.reshape(B, C, H, W)
